# revision 1
# baseline (speedup 1.0000x reference)
"""Trainium2 Bass kernel for nn_Agent (5-GRU actor-critic encoder + value MLP).

Strategy
--------
Data-parallel over batch: B=2048 split as 256 per core across 8 cores.
On-chip layout is feature-on-partitions, batch-on-free: every GRU state is an
SBUF tile [H, 256] and every gate pre-activation is a PSUM tile produced by
accumulating ih and hh matmuls directly in PSUM (out = lhsT.T @ rhs with
K = feature dim on partitions).

The three small GRUs (oo H=64, pa H=32, rr H=32) are packed into ONE
128-partition "small lane" via block-diagonal merged weights built on the
host; since a matmul's duration is set by the moving free dim (N) and not K,
the zero padding costs nothing and the whole small-lane hh update is a single
matmul per gate.

The mx GRU consumes the small lane's packed state directly (its input is
concat(a,r,o) up to a row permutation folded into the mx input weights on
host). All matmul operands and elementwise tiles are bf16 (PSUM fp32).
prev_action and reward are host-stacked into one K=65 input so the reward's
rank-1 projection rides inside the prev_action matmul for free (contraction
depth is free on the PE; only streamed rows cost time). The small lane's r/z
input projections are batched two timesteps per matmul (N=512) into one
2-bank PSUM tile whose per-step r|z halves feed a single strided-AP sigmoid;
the oa lane runs per-step with a merged r|z sigmoid and a double-buffered r/z
PSUM tile so its h-independent projection matmuls run ahead and fill
TensorEngine gaps (PSUM slot lifetimes, not SBUF buffering, gate this
schedule). Input DMAs are fetched in 8-step chunks. The GRU update is
computed as h' = z*h - (z-1)*n so only two cheap DVE ops follow the tanh on
the recurrent critical path, with the blend terms z*h and z-1 off-chain; for
all three lanes the xn + r*hn add is accumulated on the TensorEngine via an
identity matmul so tanh reads PSUM directly.

Inputs are pre-transposed on the host ([T,B,F] -> [T,F,B] per core) so all
device DMAs are contiguous and no on-chip transposes exist anywhere.
"""

import os
import sys
import time

import numpy as np

for _p in ("/opt/trn_rl_repo", "/root/.axon_site/_ro/trn_rl_repo"):
    if _p not in sys.path and os.path.isdir(_p):
        sys.path.insert(0, _p)

import concourse.bass as bass  # noqa: E402
import concourse.mybir as mybir  # noqa: E402
import concourse.tile as tile  # noqa: E402
from concourse import bacc  # noqa: E402

F32 = mybir.dt.float32
BF16 = mybir.dt.bfloat16
F32R = mybir.dt.float32r
AFT = mybir.ActivationFunctionType
OP = mybir.AluOpType

T, B, F, A = 128, 2048, 256, 64
NCORES = 8
BL = B // NCORES  # 256 batch per core

_GATE = 128  # merged-lane gate stride in weight free dim


def _build(T_steps: int, num_devices: int, use_rz_bias: bool):
    """Build the Bass/Tile program. Returns the compiled-ready Bacc."""
    nc = bacc.Bacc("TRN2", target_bir_lowering=False, debug=False,
                   num_devices=num_devices)

    f32 = F32
    # ---- DRAM I/O ----
    obsT = nc.dram_tensor("obsT", [T_steps, F, BL], BF16, kind="ExternalInput")
    actT = nc.dram_tensor("actT", [T_steps, A, BL], BF16, kind="ExternalInput")
    prT = nc.dram_tensor("prT", [T_steps, A + 1, BL], BF16, kind="ExternalInput")

    wd = {}
    for name, shape in [
        ("woa_hh", [128, 384]), ("woa_obs", [256, 384]), ("woa_act", [64, 384]),
        ("wmx_hh", [128, 384]), ("wmx_ih", [128, 384]),
        ("wsm_hh", [128, 384]), ("wsm_obs", [256, 384]),
        ("wsm_pr", [65, 384]),
        ("bnh_oa", [128]), ("bnh_sm", [128]), ("bnh_mx", [128]),
        ("bni_oa", [128]), ("bni_sm", [128]), ("bni_mx", [128]),
        ("w1t", [256, 256]), ("b1", [256]), ("w2t", [256, 1]), ("b2", [1]),
        ("wident", [128, 128]),
    ]:
        dt = BF16 if name.startswith("w") else f32
        wd[name] = nc.dram_tensor(name, shape, dt, kind="ExternalInput")
    if use_rz_bias:
        for name in ("brz_oa", "brz_sm", "brz_mx"):
            wd[name] = nc.dram_tensor(name, [128, 2], f32, kind="ExternalInput")

    val = nc.dram_tensor("val", [1, BL], f32, kind="ExternalOutput")

    def mm_acc(psum_ap, pairs):
        n = len(pairs)
        for i, (w, x) in enumerate(pairs):
            nc.tensor.matmul(psum_ap, w, x,
                             start=(i == 0), stop=(i == n - 1))

    with tile.TileContext(nc) as tc:
        with (
            tc.tile_pool(name="const", bufs=1) as cp,
            tc.tile_pool(name="io", bufs=4) as iop,
            tc.tile_pool(name="psum_pair", bufs=1, space="PSUM") as ppp,
            tc.tile_pool(name="psum", bufs=1, space="PSUM") as pp,
            tc.tile_pool(name="psum_nh2", bufs=2, space="PSUM") as pp2,
            tc.tile_pool(name="tmp", bufs=3) as tp,
            tc.tile_pool(name="state", bufs=4) as hp,
        ):
            # ---- prefetch first input chunk before weight loads ----
            nch0 = min(8, T_steps)
            obs_c0 = [iop.tile([128, nch0, BL], BF16, tag=f"obs{k}",
                               name=f"obs{k}_pre") for k in range(2)]
            for k in range(2):
                nc.sync.dma_start(
                    obs_c0[k],
                    obsT[0:nch0, k * 128:(k + 1) * 128].rearrange("t p b -> p t b"))
            act_c0 = iop.tile([64, nch0, BL], BF16, tag="act", name="act_pre")
            nc.sync.dma_start(act_c0, actT[0:nch0].rearrange("t p b -> p t b"))
            pr_c0 = iop.tile([65, nch0, BL], BF16, tag="pr", name="pr_pre")
            nc.sync.dma_start(pr_c0, prT[0:nch0].rearrange("t p b -> p t b"))

            # ---- load weights ----
            def cload(name, shape, src_ap, dt=BF16):
                t = cp.tile(shape, dt, name=f"c_{name}")
                nc.sync.dma_start(t, src_ap)
                return t

            woa_hh = cload("woa_hh", [128, 384], wd["woa_hh"][:])
            woa_obs = cload("woa_obs", [128, 2, 384],
                            wd["woa_obs"][:].rearrange("(o p) m -> p o m", p=128))
            woa_act = cload("woa_act", [64, 384], wd["woa_act"][:])
            wmx_hh = cload("wmx_hh", [128, 384], wd["wmx_hh"][:])
            wmx_ih = cload("wmx_ih", [128, 384], wd["wmx_ih"][:])
            wsm_hh = cload("wsm_hh", [128, 384], wd["wsm_hh"][:])
            wsm_obs = cload("wsm_obs", [128, 2, 384],
                            wd["wsm_obs"][:].rearrange("(o p) m -> p o m", p=128))
            wsm_pr = cload("wsm_pr", [65, 384], wd["wsm_pr"][:])
            bnh = {k: cload(f"bnh_{k}", [128, 1], wd[f"bnh_{k}"][:][:, None], dt=f32)
                   for k in ("oa", "sm", "mx")}
            bni = {k: cload(f"bni_{k}", [128, 1], wd[f"bni_{k}"][:][:, None], dt=f32)
                   for k in ("oa", "sm", "mx")}
            brz = {}
            if use_rz_bias:
                brz = {k: cload(f"brz_{k}", [128, 2], wd[f"brz_{k}"][:], dt=f32)
                       for k in ("oa", "sm", "mx")}
            ident = cload("wident", [128, 128], wd["wident"][:])

            # ---- init states ----
            h_oa = hp.tile([128, BL], BF16, tag="h_oa", name="h_oa_init")
            h_sm = hp.tile([128, BL], BF16, tag="h_sm", name="h_sm_init")
            h_mx = hp.tile([128, BL], BF16, tag="h_mx", name="h_mx_init")
            for h in (h_oa, h_sm, h_mx):
                nc.vector.memset(h, 0.0)

            GS = [slice(g * _GATE, (g + 1) * _GATE) for g in range(3)]

            def gru_elem(pfx, t_idx, p_r, p_z, p_nh, h_old, merged_sig=False):
                """Gate math given complete pre-activation psums. Returns h_new.
                Critical chain after tanh: u = w*n (DVE), h' = v-u (DVE); the
                blend terms v = z*h and w = z-1 run off-chain on Pool."""
                rz_s = tp.tile([128, 512], BF16, tag=f"{pfx}_rzs",
                               name=f"{pfx}_rzs_{t_idx}")
                if merged_sig and not use_rz_bias:
                    nc.scalar.activation(rz_s, p_r, AFT.Sigmoid)
                else:
                    if merged_sig:
                        # p_r carries both gate halves; recover the r half
                        r_ap = p_r[:, 0] if len(p_r.shape) == 3 else p_r[:, 0:256]
                    else:
                        r_ap = p_r
                    bkw = ({"bias": brz[pfx][:, 0:1]} if use_rz_bias else {})
                    nc.scalar.activation(rz_s[:, 0:256], r_ap, AFT.Sigmoid, **bkw)
                    bkw = ({"bias": brz[pfx][:, 1:2]} if use_rz_bias else {})
                    nc.scalar.activation(rz_s[:, 256:512], p_z, AFT.Sigmoid, **bkw)
                v_s = tp.tile([128, BL], BF16, tag=f"{pfx}_v", name=f"{pfx}_v_{t_idx}")
                v_eng = nc.vector if pfx == "mx" else nc.gpsimd
                v_eng.tensor_mul(v_s, rz_s[:, 256:512], h_old)
                w_s = tp.tile([128, BL], BF16, tag=f"{pfx}_w", name=f"{pfx}_w_{t_idx}")
                if pfx == "mx":
                    nc.vector.tensor_scalar_sub(w_s, rz_s[:, 256:512], 1.0)
                else:
                    nc.gpsimd.tensor_scalar_sub(w_s, rz_s[:, 256:512], 1.0)
                t1 = tp.tile([128, BL], BF16, tag=f"{pfx}_t1", name=f"{pfx}_t1_{t_idx}")
                nc.vector.scalar_tensor_tensor(t1, p_nh[:, 256:512], bnh[pfx],
                                               rz_s[:, 0:256], OP.add, OP.mult)
                n_s = tp.tile([128, BL], BF16, tag=f"{pfx}_n", name=f"{pfx}_n_{t_idx}")
                if True:
                    # accumulate t1 onto the xn psum region on the PE, then
                    # tanh straight from PSUM
                    nc.tensor.matmul(p_nh[:, 0:256], ident, t1,
                                     start=False, stop=True, skip_group_check=True)
                    nc.scalar.activation(n_s, p_nh[:, 0:256], AFT.Tanh,
                                         bias=bni[pfx])
                u_s = tp.tile([128, BL], BF16, tag=f"{pfx}_u", name=f"{pfx}_u_{t_idx}")
                nc.vector.tensor_mul(u_s, w_s, n_s)
                h_new = hp.tile([128, BL], BF16, tag=f"h_{pfx}", name=f"h_{pfx}_{t_idx}")
                nc.vector.tensor_sub(h_new, v_s, u_s)
                return h_new

            CH = 8  # DMA chunk: steps fetched per DMA set
            assert T_steps % 2 == 0
            obs_c = act_c = pr_c = None
            for t0 in range(0, T_steps, 2):
                pi = t0 // 2
                if t0 % CH == 0:
                    nch = min(CH, T_steps - t0)
                    ci = t0 // CH
                    if ci == 0:
                        obs_c, act_c, pr_c = obs_c0, act_c0, pr_c0
                    else:
                        obs_c = [iop.tile([128, nch, BL], BF16, tag=f"obs{k}",
                                          name=f"obs{k}_{ci}") for k in range(2)]
                        for k in range(2):
                            nc.sync.dma_start(
                                obs_c[k],
                                obsT[t0:t0 + nch, k * 128:(k + 1) * 128].rearrange(
                                    "t p b -> p t b"))
                        act_c = iop.tile([64, nch, BL], BF16, tag="act",
                                         name=f"act_{ci}")
                        nc.sync.dma_start(
                            act_c, actT[t0:t0 + nch].rearrange("t p b -> p t b"))
                        pr_c = iop.tile([65, nch, BL], BF16, tag="pr",
                                        name=f"pr_{ci}")
                        nc.sync.dma_start(
                            pr_c, prT[t0:t0 + nch].rearrange("t p b -> p t b"))
                sc = t0 % CH
                obs_p = None  # chunk slices used below

                # pair-batched r/z input projections at N=512 (both steps at once)
                ob0 = obs_c[0][:, sc:sc + 2]
                ob1 = obs_c[1][:, sc:sc + 2]
                ap_act = act_c[:, sc:sc + 2]
                pr2 = pr_c[:, sc:sc + 2]
                p_sm_pair = ppp.tile([128, 1024], f32, tag="sm_rzp",
                                     name=f"sm_rzp_{pi}")
                prs = {"sm": (p_sm_pair[:, 0:512], p_sm_pair[:, 512:1024])}
                for g in (0, 1):
                    psm = prs["sm"][g]
                    gsl = GS[g]
                    mm = nc.tensor.matmul
                    mm(psm, wsm_obs[:, 0][..., gsl], ob0, start=True, stop=False,
                       skip_group_check=True)
                    mm(psm, wsm_obs[:, 1][..., gsl], ob1, start=False, stop=False,
                       skip_group_check=True)
                    mm(psm, wsm_pr[:, gsl], pr2, start=False, stop=False,
                       skip_group_check=True)

                for s in (0, 1):
                    t = t0 + s
                    sl = slice(s * 256, (s + 1) * 256)
                    obs_s = [obs_c[0][:, sc + s], obs_c[1][:, sc + s]]

                    # ---- small lane step ----
                    p_r, p_z = prs["sm"]
                    sm_sig_in = p_sm_pair.rearrange(
                        "p (g t b) -> p g t b", g=2, t=2)[:, :, s]
                    hh = wsm_hh
                    nc.tensor.matmul(p_r[:, sl], hh[:, GS[0]], h_sm,
                                     start=False, stop=(s == 1),
                                     skip_group_check=True)
                    nc.tensor.matmul(p_z[:, sl], hh[:, GS[1]], h_sm,
                                     start=False, stop=(s == 1),
                                     skip_group_check=True)
                    p_nh = pp.tile([128, 512], f32, tag="sm_nh", name=f"sm_nh_{t}")
                    nc.tensor.matmul(p_nh[:, 256:512], hh[:, GS[2]], h_sm,
                                     start=True, stop=True)
                    xn = [(wsm_obs[:, 0][..., GS[2]], obs_s[0]),
                          (wsm_obs[:, 1][..., GS[2]], obs_s[1]),
                          (wsm_pr[:, GS[2]], pr_c[:, sc + s])]
                    for i, (w, x) in enumerate(xn):
                        nc.tensor.matmul(p_nh[:, 0:256], w, x,
                                         start=(i == 0), stop=False,
                                         skip_group_check=True)
                    h_sm = gru_elem("sm", t, sm_sig_in, p_z[:, sl], p_nh, h_sm,
                                    merged_sig=True)

                    # ---- oa lane step ----
                    p_rz_oa = pp2.tile([128, 512], f32, tag="oa_rz", name=f"oa_rz_{t}")
                    for g, psl in ((0, slice(0, 256)), (1, slice(256, 512))):
                        ihs = [(woa_obs[:, 0][..., GS[g]], obs_s[0]),
                               (woa_obs[:, 1][..., GS[g]], obs_s[1]),
                               (woa_act[:, GS[g]], act_c[:, sc + s]),
                               (woa_hh[:, GS[g]], h_oa)]
                        for i, (wt, x) in enumerate(ihs):
                            nc.tensor.matmul(p_rz_oa[:, psl], wt, x,
                                             start=(i == 0), stop=(i == 3))
                    p_nh = pp.tile([128, 512], f32, tag="oa_nh", name=f"oa_nh_{t}")
                    nc.tensor.matmul(p_nh[:, 256:512], woa_hh[:, GS[2]], h_oa,
                                     start=True, stop=True)
                    xn = [(woa_obs[:, 0][..., GS[2]], obs_s[0]),
                          (woa_obs[:, 1][..., GS[2]], obs_s[1]),
                          (woa_act[:, GS[2]], act_c[:, sc + s])]
                    for i, (w, x) in enumerate(xn):
                        nc.tensor.matmul(p_nh[:, 0:256], w, x,
                                         start=(i == 0), stop=False,
                                         skip_group_check=True)
                    h_oa = gru_elem("oa", t, p_rz_oa, p_rz_oa[:, 256:512], p_nh, h_oa,
                                    merged_sig=True)
                    # ---- mx lane step (consumes fresh h_sm) ----
                    p_rz = pp.tile([128, 512], f32, tag="mx_rz", name=f"mx_rz_{t}")
                    p_nh2 = pp.tile([128, 512], f32, tag="mx_nh", name=f"mx_nh_{t}")
                    for g, psl in ((0, slice(0, 256)), (1, slice(256, 512))):
                        nc.tensor.matmul(p_rz[:, psl], wmx_hh[:, GS[g]], h_mx,
                                         start=True, stop=False)
                        nc.tensor.matmul(p_rz[:, psl], wmx_ih[:, GS[g]], h_sm,
                                         start=False, stop=True)
                    nc.tensor.matmul(p_nh2[:, 256:512], wmx_hh[:, GS[2]], h_mx,
                                     start=True, stop=True)
                    nc.tensor.matmul(p_nh2[:, 0:256], wmx_ih[:, GS[2]], h_sm,
                                     start=True, stop=False,
                                     skip_group_check=True)
                    h_mx = gru_elem("mx", t, p_rz, p_rz[:, 256:512],
                                    p_nh2, h_mx, merged_sig=True)


            # ---- value MLP on last states: feat = [h_oa; h_mx] ----
            w1t = cload("w1t", [128, 2, 256],
                        wd["w1t"][:].rearrange("(o p) m -> p o m", p=128))
            b1 = cload("b1", [128, 2], wd["b1"][:].rearrange("(o p) -> p o", p=128), dt=f32)
            w2t = cload("w2t", [128, 2, 1],
                        wd["w2t"][:].rearrange("(o p) m -> p o m", p=128))
            b2 = cload("b2", [1, 1], wd["b2"][:][:, None], dt=f32)
            h1 = []
            for m in range(2):
                p = pp.tile([128, BL], f32, tag=("oa_nh", "sm_nh")[m], name=f"p_h1_{m}")
                ms = slice(m * 128, (m + 1) * 128)
                nc.tensor.matmul(p, w1t[:, 0, ms], h_oa, start=True, stop=False)
                nc.tensor.matmul(p, w1t[:, 1, ms], h_mx, start=False, stop=True)
                h = tp.tile([128, BL], BF16, tag=f"h1_{m}", name=f"h1_{m}")
                nc.scalar.activation(h, p, AFT.Tanh, bias=b1[:, m:m + 1])
                h1.append(h)
            p_val = pp.tile([1, BL], f32, tag="mx_rz", name="p_val")
            nc.tensor.matmul(p_val, w2t[:, 0], h1[0], start=True, stop=False)
            nc.tensor.matmul(p_val, w2t[:, 1], h1[1], start=False, stop=True)
            out_s = tp.tile([1, BL], f32, tag="out", name="out_s")
            nc.scalar.activation(out_s, p_val, AFT.Identity, bias=b2[0:1, 0:1])
            nc.sync.dma_start(val[:], out_s)

    nc.compile()
    return nc


def _prep_weights(inp: dict) -> dict:
    """Host-side: transpose/merge weights into the lhsT layouts the kernel uses."""
    f4 = np.float32
    g = lambda w, i: np.asarray(w)[i * (w.shape[0] // 3):(i + 1) * (w.shape[0] // 3), :]
    out = {}
    out["woa_hh"] = np.ascontiguousarray(np.asarray(inp["oa_whh"]).T, f4)
    wih_oa_t = np.asarray(inp["oa_wih"]).T  # [320, 384]
    out["woa_obs"] = np.ascontiguousarray(wih_oa_t[0:256], f4)
    out["woa_act"] = np.ascontiguousarray(wih_oa_t[256:320], f4)
    out["wmx_hh"] = np.ascontiguousarray(np.asarray(inp["mx_whh"]).T, f4)
    # h_small rows = [oo(0:64), pa(64:96), rr(96:128)]; mx input = [a=pa, r=rr, o=oo]
    perm = np.concatenate([np.arange(64, 128), np.arange(0, 32), np.arange(32, 64)])
    out["wmx_ih"] = np.ascontiguousarray(np.asarray(inp["mx_wih"]).T[perm], f4)

    wsm_hh = np.zeros((128, 384), f4)
    wsm_obs = np.zeros((256, 384), f4)
    wsm_pr = np.zeros((65, 384), f4)
    for gi in range(3):
        c = _GATE * gi
        wsm_hh[0:64, c + 0:c + 64] = g(inp["oo_whh"], gi).T
        wsm_hh[64:96, c + 64:c + 96] = g(inp["pa_whh"], gi).T
        wsm_hh[96:128, c + 96:c + 128] = g(inp["rr_whh"], gi).T
        wsm_obs[:, c + 0:c + 64] = g(inp["oo_wih"], gi).T
        wsm_pr[0:64, c + 64:c + 96] = g(inp["pa_wih"], gi).T
        wsm_pr[64:65, c + 96:c + 128] = g(inp["rr_wih"], gi).T
    out["wsm_hh"], out["wsm_obs"] = wsm_hh, wsm_obs
    out["wsm_pr"] = wsm_pr

    def pack_small(v_oo, v_pa, v_rr):
        r = np.zeros(128, f4)
        r[0:64], r[64:96], r[96:128] = v_oo, v_pa, v_rr
        return r

    for key, pfx in (("oa", "oa"), ("mx", "mx")):
        bih, bhh = np.asarray(inp[f"{key}_bih"]), np.asarray(inp[f"{key}_bhh"])
        H = bih.shape[0] // 3
        out[f"bnh_{pfx}"] = np.ascontiguousarray(bhh[2 * H:3 * H], f4)
        out[f"bni_{pfx}"] = np.ascontiguousarray(bih[2 * H:3 * H], f4)
        out[f"brz_{pfx}"] = np.ascontiguousarray(
            np.stack([bih[0:H] + bhh[0:H], bih[H:2 * H] + bhh[H:2 * H]], 1), f4)
    bsm = {}
    for part in ("bih", "bhh"):
        vs = {k: np.asarray(inp[f"{k}_{part}"]) for k in ("oo", "pa", "rr")}
        bsm[part] = [pack_small(vs["oo"][64 * gi:64 * (gi + 1)],
                                vs["pa"][32 * gi:32 * (gi + 1)],
                                vs["rr"][32 * gi:32 * (gi + 1)]) for gi in range(3)]
    out["bnh_sm"] = bsm["bhh"][2]
    out["bni_sm"] = bsm["bih"][2]
    out["brz_sm"] = np.ascontiguousarray(
        np.stack([bsm["bih"][0] + bsm["bhh"][0], bsm["bih"][1] + bsm["bhh"][1]], 1), f4)

    out["w1t"] = np.ascontiguousarray(np.asarray(inp["W1"]).T, f4)
    out["b1"] = np.ascontiguousarray(np.asarray(inp["b1"]), f4)
    out["w2t"] = np.ascontiguousarray(np.asarray(inp["W2"]).T, f4)
    out["b2"] = np.ascontiguousarray(np.asarray(inp["b2"]), f4)
    out["wident"] = np.eye(128, dtype=f4)
    import ml_dtypes
    for k in list(out):
        if k.startswith("w"):
            out[k] = out[k].astype(ml_dtypes.bfloat16)
    return out


def _prep_core_inputs(inp: dict, w: dict, c: int, T_steps: int, use_rz_bias: bool):
    bs = slice(c * BL, (c + 1) * BL)
    import ml_dtypes
    f4 = ml_dtypes.bfloat16
    m = {
        "obsT": np.ascontiguousarray(
            np.asarray(inp["obs"])[:T_steps, bs, :].transpose(0, 2, 1), f4),
        "actT": np.ascontiguousarray(
            np.asarray(inp["action"])[:T_steps, bs, :].transpose(0, 2, 1), f4),
        "prT": np.ascontiguousarray(np.concatenate([
            np.asarray(inp["prev_action"])[:T_steps, bs, :].transpose(0, 2, 1),
            np.asarray(inp["reward"])[:T_steps, bs, :].transpose(0, 2, 1),
        ], axis=1), f4),
    }
    for k, v in w.items():
        if not use_rz_bias and k.startswith("brz_"):
            continue
        m[k] = v
    return m


_RUN_KW = {}  # test harness can set trace=True here


def run(inputs: dict, T_steps: int = T, n_cores: int = NCORES):
    from concourse import bass_utils
    from concourse.bass_interp import get_hw_module

    w = _prep_weights(inputs)
    use_rz_bias = any(
        np.abs(w[f"brz_{k}"]).max() > 0 for k in ("oa", "sm", "mx"))
    nc = _build(T_steps, n_cores, use_rz_bias)
    nc.m = get_hw_module(nc.m)
    in_maps = [_prep_core_inputs(inputs, w, c, T_steps, use_rz_bias)
               for c in range(n_cores)]
    res = bass_utils.run_bass_kernel_spmd(
        nc, in_maps, core_ids=list(range(n_cores)), **_RUN_KW)
    vals = [res.results[c]["val"].reshape(BL) for c in range(n_cores)]
    out = np.concatenate(vals).astype(np.float32).reshape(-1, 1)
    run.last_result = res
    return out


def run_timed(inputs: dict, iters: int = 4, T_steps: int = T,
              n_cores: int = NCORES):
    """Like run(), but keeps the compiled executable and wall-clocks repeated
    executions with device-resident inputs. Returns (out, times_sec)."""
    import jax
    from concourse import bass2jax
    from concourse.bass_interp import get_hw_module

    w = _prep_weights(inputs)
    use_rz_bias = any(np.abs(w[f"brz_{k}"]).max() > 0 for k in ("oa", "sm", "mx"))
    nc = _build(T_steps, n_cores, use_rz_bias)
    nc.m = get_hw_module(nc.m)
    in_maps = [_prep_core_inputs(inputs, w, c, T_steps, use_rz_bias)
               for c in range(n_cores)]

    bass2jax.install_neuronx_cc_hook()
    partition_name = nc.partition_id_tensor.name if nc.partition_id_tensor else None
    in_names, out_names, out_avals, zero_outs = [], [], [], []
    import concourse.mybir as _my
    for alloc in nc.m.functions[0].allocations:
        if not isinstance(alloc, _my.MemoryLocationSet):
            continue
        name = alloc.memorylocations[0].name
        if alloc.kind == "ExternalInput":
            if name != partition_name:
                in_names.append(name)
        elif alloc.kind == "ExternalOutput":
            shape = tuple(alloc.tensor_shape)
            dtype = _my.dt.np(alloc.dtype)
            out_names.append(name)
            out_avals.append(jax.core.ShapedArray(shape, dtype))
            zero_outs.append(np.zeros(shape, dtype))
    n_params = len(in_names)
    all_in = list(in_names) + list(out_names)
    if partition_name is not None:
        all_in.append(partition_name)

    def _body(*args):
        operands = list(args)
        if partition_name is not None:
            operands.append(bass2jax.partition_id_tensor())
        outs = bass2jax._bass_exec_p.bind(
            *operands, out_avals=tuple(out_avals), in_names=tuple(all_in),
            out_names=tuple(out_names), lowering_input_output_aliases=(),
            sim_require_finite=True, sim_require_nnan=True, nc=nc)
        return tuple(outs)

    devices = jax.devices()[:n_cores]
    mesh = bass2jax.Mesh(np.asarray(devices), ("core",))
    donate = tuple(range(n_params, n_params + len(out_names)))
    sharded = jax.jit(
        bass2jax.shard_map(_body, mesh=mesh,
                           in_specs=(bass2jax.PartitionSpec("core"),) * (n_params + len(out_names)),
                           out_specs=(bass2jax.PartitionSpec("core"),) * len(out_names),
                           check_rep=False),
        donate_argnums=donate, keep_unused=True)

    concat_in = [np.concatenate([np.asarray(in_maps[c][nm]) for c in range(n_cores)], axis=0)
                 for nm in in_names]
    sh = jax.sharding.NamedSharding(mesh, bass2jax.PartitionSpec("core"))
    dev_in = [jax.device_put(x, sh) for x in concat_in]

    def zeros():
        return [jax.device_put(np.zeros((n_cores * z.shape[0], *z.shape[1:]), z.dtype), sh)
                for z in zero_outs]

    times = []
    out_arrs = None
    for _ in range(iters):
        zs = zeros()
        jax.block_until_ready(zs)
        t0 = time.time()
        out_arrs = sharded(*dev_in, *zs)
        jax.block_until_ready(out_arrs)
        times.append(time.time() - t0)

    res = {name: np.asarray(out_arrs[i]).reshape(n_cores, *out_avals[i].shape)
           for i, name in enumerate(out_names)}
    vals = [res["val"][c].reshape(BL) for c in range(n_cores)]
    out = np.concatenate(vals).astype(np.float32).reshape(-1, 1)
    return out, times


def kernel(**inputs) -> np.ndarray:
    return run(inputs)

